# revision 3
# baseline (speedup 1.0000x reference)
"""Trainium2 Bass kernel for nn_CustomLoss (CrossEntropy + binary-remap BCE).

loss = mean_i[ logsumexp(pred_i) - pred_i[t_i] ]
     + 100 * mean_i[ 1{ LUT[argmax(pred_i)] != LUT[t_i] } ]

with LUT = [0,0,1,1,1,1,1,1,0,0]  (LUT[j] = 1 iff 2 <= j <= 7).

Sharding: data-parallel over the batch axis across 8 NeuronCores.  The host
additionally CLASS-BUCKETS the rows: all rows with target class c are placed
in a fixed 200-column bucket of the per-core [128, 10, 2000] class-major
layout (row (p, w) has its 10 logits at [p, :, w]; bucket c = w in
[200c, 200c+200)).  Bucketing is a pure row permutation plus padding with
all-zero rows; it makes both target-dependent reductions data-independent:

  * CE gather:  sum_rows pred[t] = sum_c (column-c sum over bucket c)
                -> 10 plain strided reduces, no scan/gather op.
  * BCE remap:  bt = LUT[t] is constant per bucket, so the mismatch count
                1{(m6 > m4) != bt} becomes a fused tensor_tensor_reduce
                (is_gt, add) with operand order swapped per bucket sign.
                Running both is_gt and is_ge and averaging gives exact
                half-weight tie handling vs the bf16 rounding ties.

Everything streams as bf16 (halves DMA and doubles DVE throughput):
  DMA   : 5 tiles [128, 10*400] bf16 per core (5.12 MB vs 11.0 MB before)
  ACT   : E = exp(x) on [128, 10, w]; ln(s) with per-partition accumulate
  DVE   : bf16 2x tensor_tensor trees: row-sum 5+5 -> 2+2+1 -> 1,
          mid6/out4 max trees, 2 fused mismatch-count TTRs, 2 column sums
  GPSIMD/PE: idle
"""

import numpy as np

# ---------------------------------------------------------------- constants
N = 2_000_000
C = 10
N_CORES = 8
P = 128
BUCKET_COLS = 200                 # per-class bucket width (cols per partition)
W_CORE = BUCKET_COLS * C          # 2000
ROWS_CORE_PAD = P * W_CORE        # 256,000 rows per core incl. pads
TILE_W = 400                      # 2 buckets per tile
N_TILES = W_CORE // TILE_W        # 5
# bt of the two buckets in each tile (classes 2i, 2i+1); LUT = 1 iff 2<=c<=7
TILE_BT = [0, 1, 1, 1, 0]
N_PADS = N_CORES * ROWS_CORE_PAD - N  # 48,000 all-zero pad rows

_CACHE = {}


# ------------------------------------------------------------- device build
def _build_nc():
    import concourse.tile as tile
    from concourse import bacc, mybir

    f32 = mybir.dt.float32
    bf16 = mybir.dt.bfloat16
    A = mybir.ActivationFunctionType
    X = mybir.AxisListType.X
    alu = mybir.AluOpType

    nc = bacc.Bacc("TRN2", target_bir_lowering=False, debug=False,
                   num_devices=N_CORES)
    comb_ds = [
        nc.dram_tensor(f"comb{i}", [P, TILE_W * C], bf16,
                       kind="ExternalInput").ap()
        for i in range(N_TILES)
    ]
    out_d = nc.dram_tensor("out", [P, 5], f32, kind="ExternalOutput").ap()

    with tile.TileContext(nc) as tc:
        with (
            tc.tile_pool(name="io", bufs=3) as io,
            tc.tile_pool(name="ep", bufs=2) as ep,
            tc.tile_pool(name="wp", bufs=2) as wp,
            tc.tile_pool(name="cp", bufs=1) as cp,
        ):
            # acc rows: 0=ln(s), 1=sum x_t (bucket lo), 2=sum x_t (bucket hi),
            #           3=count gt, 4=count ge
            acc = cp.tile([P, 5, N_TILES], f32)
            w = TILE_W

            for i in range(N_TILES):
                ct = io.tile([P, C * w], bf16, tag="comb")
                nc.sync.dma_start(ct[:], comb_ds[i])
                cv = ct[:].rearrange("p (c w) -> p c w", c=C)

                # ---- CE: exp, row-sum tree, ln-accumulate
                et = ep.tile([P, C, w], bf16, tag="E")
                nc.scalar.activation(et[:], cv, A.Exp)

                l1 = wp.tile([P, 5, w], bf16, tag="l1")
                nc.vector.tensor_tensor(l1[:], et[:, 0:5, :], et[:, 5:10, :],
                                        op=alu.add)
                l2 = wp.tile([P, 2, w], bf16, tag="l2")
                nc.vector.tensor_tensor(l2[:], l1[:, 0:2, :], l1[:, 2:4, :],
                                        op=alu.add)
                l3 = wp.tile([P, w], bf16, tag="l3")
                nc.vector.tensor_tensor(l3[:], l2[:, 0, :], l2[:, 1, :],
                                        op=alu.add)
                s = wp.tile([P, w], bf16, tag="s")
                nc.vector.tensor_tensor(s[:], l3[:], l1[:, 4, :], op=alu.add)

                lns = wp.tile([P, w], bf16, tag="lns")
                nc.scalar.activation(lns[:], s[:], A.Ln,
                                     accum_out=acc[:, 0, i:i + 1])

                # ---- CE gather: column-c sums over the tile's two buckets
                c0 = 2 * i
                nc.vector.reduce_sum(acc[:, 1, i:i + 1],
                                     cv[:, c0, 0:BUCKET_COLS], axis=X)
                nc.vector.reduce_sum(acc[:, 2, i:i + 1],
                                     cv[:, c0 + 1, BUCKET_COLS:w], axis=X)

                # ---- BCE: mid6/out4 max trees
                m1 = wp.tile([P, 3, w], bf16, tag="m1")
                nc.vector.tensor_tensor(m1[:], cv[:, 2:5, :], cv[:, 5:8, :],
                                        op=alu.max)
                m2 = wp.tile([P, w], bf16, tag="m2")
                nc.vector.tensor_tensor(m2[:], m1[:, 0, :], m1[:, 1, :],
                                        op=alu.max)
                m6 = wp.tile([P, w], bf16, tag="m6")
                nc.vector.tensor_tensor(m6[:], m2[:], m1[:, 2, :], op=alu.max)
                o1 = wp.tile([P, 2, w], bf16, tag="o1")
                nc.vector.tensor_tensor(o1[:], cv[:, 0:2, :], cv[:, 8:10, :],
                                        op=alu.max)
                m4 = wp.tile([P, w], bf16, tag="m4")
                nc.vector.tensor_tensor(m4[:], o1[:, 0, :], o1[:, 1, :],
                                        op=alu.max)

                # mismatch: bt=0 -> count m6 >(=) m4 ; bt=1 -> count m4 >(=) m6
                a, b = (m6, m4) if TILE_BT[i] == 0 else (m4, m6)
                mo = wp.tile([P, w], bf16, tag="mo")
                nc.vector.scalar_tensor_tensor(
                    mo[:], a[:], 0.0, b[:], op0=alu.bypass, op1=alu.is_gt,
                    accum_out=acc[:, 3, i:i + 1])
                mo2 = wp.tile([P, w], bf16, tag="mo2")
                nc.vector.scalar_tensor_tensor(
                    mo2[:], a[:], 0.0, b[:], op0=alu.bypass, op1=alu.is_ge,
                    accum_out=acc[:, 4, i:i + 1])

            # ---- final per-partition fold + store
            out_t = cp.tile([P, 5], f32)
            nc.vector.reduce_sum(out_t[:], acc[:], axis=X)
            nc.sync.dma_start(out_d[:], out_t[:])

    # Force a single activation table containing both Exp and Ln so the
    # compiler does not ping-pong ACT_TABLE_LOADs.
    import concourse.bacc as bacc_mod
    from concourse.hw_specs import get_activation_tables
    orig = get_activation_tables(nc.m.arch)
    combined = None
    for k, v in orig.items():
        if (mybir.ActivationFunctionType.Exp in v
                and mybir.ActivationFunctionType.Ln in v):
            combined = k
            break
    if combined is not None:
        patched = {k: (v if k == combined else set()) for k, v in orig.items()}
        saved = bacc_mod.get_activation_tables
        bacc_mod.get_activation_tables = lambda arch: patched
        try:
            nc.compile()
        finally:
            bacc_mod.get_activation_tables = saved
    else:
        nc.compile()
    return nc


def _get_nc():
    if "nc" not in _CACHE:
        _CACHE["nc"] = _build_nc()
    return _CACHE["nc"]


# ------------------------------------------------------------------- host
def _host_prep(pred, target):
    """Class-bucketed shard/pack: bf16 tiles [P, 10, TILE_W] per core."""
    import ml_dtypes

    pred = np.asarray(pred)
    if pred.dtype != ml_dtypes.bfloat16:
        pred = pred.astype(np.float32).astype(ml_dtypes.bfloat16)
    target = np.asarray(target).astype(np.int32)

    order = np.argsort(target, kind="stable")
    counts = np.bincount(target, minlength=C)
    offs = np.zeros(C + 1, np.int64)
    offs[1:] = np.cumsum(counts)

    in_maps = []
    for k in range(N_CORES):
        R = np.full((C, BUCKET_COLS * P), -1, np.int64)
        for c in range(C):
            cnt = int(counts[c])
            base, rem = divmod(cnt, N_CORES)
            share = base + (1 if k < rem else 0)
            assert share <= BUCKET_COLS * P, (
                f"class {c} overflow on core {k}: {share}")
            start = offs[c] + k * base + min(k, rem)
            R[c, :share] = order[start:start + share]
        # [C, P*200] -> [C, P, 200] -> [P, C, 200] -> [P, W_CORE]
        Rpw = R.reshape(C, P, BUCKET_COLS).transpose(1, 0, 2)

        flat = Rpw.reshape(-1)
        Xg = pred[np.where(flat >= 0, flat, 0)]
        Xg[flat < 0] = ml_dtypes.bfloat16(0.0)
        # [P, C_bucket, 200, C_class] -> [P, C_class, C_bucket*200]
        Xc = Xg.reshape(P, C, BUCKET_COLS, C).transpose(0, 3, 1, 2) \
               .reshape(P, C, W_CORE)

        m = {}
        for i in range(N_TILES):
            sl = Xc[:, :, i * TILE_W:(i + 1) * TILE_W]
            m[f"comb{i}"] = np.ascontiguousarray(sl).reshape(P, C * TILE_W)
        in_maps.append(m)
    return in_maps


def kernel(pred, target):
    from concourse.bass_utils import run_bass_kernel_spmd

    nc = _get_nc()
    in_maps = _host_prep(pred, target)
    res = run_bass_kernel_spmd(nc, in_maps, core_ids=list(range(N_CORES)))

    s_ln = s_xt = s_gt = s_ge = 0.0
    for k in range(N_CORES):
        o = res.results[k]["out"].astype(np.float64)
        s_ln += o[:, 0].sum()
        s_xt += o[:, 1].sum() + o[:, 2].sum()
        s_gt += o[:, 3].sum()
        s_ge += o[:, 4].sum()

    # all-zero pad rows: s = 10 -> ln(10); x_t col adds 0; d = 0 -> gt 0, ge 1
    s_ln -= N_PADS * np.log(10.0)
    mism = 0.5 * (s_gt + s_ge) - 0.5 * N_PADS

    ce = (s_ln - s_xt) / N
    bce = 100.0 * mism / N
    return np.float32(ce + bce)


# revision 6
# speedup vs baseline: 1.1063x; 1.1063x over previous
"""Trainium2 Bass kernel for nn_CustomLoss (CrossEntropy + binary-remap BCE).

loss = mean_i[ logsumexp(pred_i) - pred_i[t_i] ]
     + 100 * mean_i[ 1{ LUT[argmax(pred_i)] != LUT[t_i] } ]

with LUT = [0,0,1,1,1,1,1,1,0,0]  (LUT[j] = 1 iff 2 <= j <= 7).

Sharding: data-parallel over the batch axis across 8 NeuronCores.  The host
additionally CLASS-BUCKETS the rows: all rows with target class c land in a
fixed 200-column bucket of the per-core [128, 10, 2000] class-major layout
(row (p, w) has its logits at [p, :, w]; bucket c = w in [200c, 200c+200)).
Bucketing is a pure row permutation plus all-zero pad rows; it makes both
target-dependent reductions data-independent:

  * CE gather:  sum_rows pred[t] = sum_c (column-c sum over bucket c)
                -> plain strided reduces, no scan/gather op.
  * BCE remap:  bt = LUT[t] is constant per tile (tiles align with bucket
                pairs of equal bt), so the mismatch count becomes
                sign(m6 - m4) summed on the Scalar engine:
                Sum sign = #gt - #lt  ->  mism = (L +- Sum)/2, which also
                half-weights bf16 ties exactly.

Everything streams as bf16.  Per-engine work (per core):
  DMA   : 5 tiles [128, 10*400] bf16 (5.12 MB)
  ACT   : exp on [128,10,w] x5, sign(d) x5, ONE ln at the end (the row-sums
          are chained into s0*s1*s2*s3*s4 so ln+accum runs once, not 5x)
  DVE   : bf16 2x tensor_tensor trees (max mid6/out4, row-sum), d = m6-m4,
          product chain, 2 strided column reduces per tile
  GPSIMD/PE: idle

Emission is phase-split (all DMA+exp first, then the DVE/ACT tail work) with
enough pool buffers that no tile's work serializes behind another's.
"""

import numpy as np

# ---------------------------------------------------------------- constants
N = 2_000_000
C = 10
N_CORES = 8
P = 128
BUCKET_COLS = 200                 # per-class bucket width (cols per partition)
W_CORE = BUCKET_COLS * C          # 2000
ROWS_CORE_PAD = P * W_CORE        # 256,000 rows per core incl. pads
TILE_W = 400                      # 2 buckets per tile
N_TILES = W_CORE // TILE_W        # 5
# bt of the two buckets in each tile (classes 2i, 2i+1); LUT = 1 iff 2<=c<=7
TILE_BT = [0, 1, 1, 1, 0]
N_PADS = N_CORES * ROWS_CORE_PAD - N  # 48,000 all-zero pad rows

_CACHE = {}


# ------------------------------------------------------------- device build
def _build_nc():
    import concourse.tile as tile
    from concourse import bacc, mybir

    f32 = mybir.dt.float32
    bf16 = mybir.dt.bfloat16
    A = mybir.ActivationFunctionType
    X = mybir.AxisListType.X
    alu = mybir.AluOpType

    nc = bacc.Bacc("TRN2", target_bir_lowering=False, debug=False,
                   num_devices=N_CORES)
    comb_ds = [
        nc.dram_tensor(f"comb{i}", [P, TILE_W * C], bf16,
                       kind="ExternalInput").ap()
        for i in range(N_TILES)
    ]
    out_d = nc.dram_tensor("out", [P, 4], f32, kind="ExternalOutput").ap()

    with tile.TileContext(nc) as tc:
        with (
            tc.tile_pool(name="io", bufs=N_TILES) as io,
            tc.tile_pool(name="ep", bufs=N_TILES) as ep,
            tc.tile_pool(name="wp", bufs=3) as wp,
            tc.tile_pool(name="pp", bufs=2) as pp,
            tc.tile_pool(name="cp", bufs=1) as cp,
        ):
            acc_b = cp.tile([P, 2, N_TILES], f32)
            acc_sg = cp.tile([P, N_TILES], f32)
            acc_ln = cp.tile([P, 1], f32)
            w = TILE_W

            # ---- phase A: all DMAs + exps
            cvs, ets = [], []
            for i in range(N_TILES):
                ct = io.tile([P, C * w], bf16, tag="comb")
                nc.sync.dma_start(ct[:], comb_ds[i])
                cv = ct[:].rearrange("p (c w) -> p c w", c=C)
                et = ep.tile([P, C, w], bf16, tag="E")
                nc.scalar.activation(et[:], cv, A.Exp)
                cvs.append(cv)
                ets.append(et)

            # ---- phase B: per-tile DVE trees + ACT sign + chained products
            prod_prev = None
            for i in range(N_TILES):
                cv, et = cvs[i], ets[i]

                # BCE max trees first so ACT's sign starts early
                m1 = wp.tile([P, 3, w], bf16, tag="m1")
                nc.vector.tensor_tensor(m1[:], cv[:, 2:5, :], cv[:, 5:8, :],
                                        op=alu.max)
                m2 = wp.tile([P, w], bf16, tag="m2")
                nc.vector.tensor_tensor(m2[:], m1[:, 0, :], m1[:, 1, :],
                                        op=alu.max)
                m6 = wp.tile([P, w], bf16, tag="m6")
                nc.vector.tensor_tensor(m6[:], m2[:], m1[:, 2, :], op=alu.max)
                o1 = wp.tile([P, 2, w], bf16, tag="o1")
                nc.vector.tensor_tensor(o1[:], cv[:, 0:2, :], cv[:, 8:10, :],
                                        op=alu.max)
                m4 = wp.tile([P, w], bf16, tag="m4")
                nc.vector.tensor_tensor(m4[:], o1[:, 0, :], o1[:, 1, :],
                                        op=alu.max)
                # orient so that sum(sign(d)) counts mismatches positively:
                # bt=0: mismatch iff m6 > m4 ; bt=1: mismatch iff m4 > m6
                da, db = (m6, m4) if TILE_BT[i] == 0 else (m4, m6)
                d = wp.tile([P, w], bf16, tag="d")
                nc.vector.tensor_tensor(d[:], da[:], db[:], op=alu.subtract)
                sg = wp.tile([P, w], bf16, tag="sg")
                nc.scalar.activation(sg[:], d[:], A.Sign,
                                     accum_out=acc_sg[:, i:i + 1])

                # CE row-sum tree
                l1 = wp.tile([P, 5, w], bf16, tag="l1")
                nc.vector.tensor_tensor(l1[:], et[:, 0:5, :], et[:, 5:10, :],
                                        op=alu.add)
                l2 = wp.tile([P, 2, w], bf16, tag="l2")
                nc.vector.tensor_tensor(l2[:], l1[:, 0:2, :], l1[:, 2:4, :],
                                        op=alu.add)
                l3 = wp.tile([P, w], bf16, tag="l3")
                nc.vector.tensor_tensor(l3[:], l2[:, 0, :], l2[:, 1, :],
                                        op=alu.add)
                s = pp.tile([P, w], bf16, tag="s")
                nc.vector.tensor_tensor(s[:], l3[:], l1[:, 4, :], op=alu.add)
                if prod_prev is None:
                    prod_prev = s
                else:
                    pr = pp.tile([P, w], bf16, tag="pr")
                    nc.vector.tensor_tensor(pr[:], prod_prev[:], s[:],
                                            op=alu.mult)
                    prod_prev = pr

                # CE gather: column-c sums over the tile's two buckets
                c0 = 2 * i
                nc.vector.reduce_sum(acc_b[:, 0, i:i + 1],
                                     cv[:, c0, 0:BUCKET_COLS], axis=X)
                nc.vector.reduce_sum(acc_b[:, 1, i:i + 1],
                                     cv[:, c0 + 1, BUCKET_COLS:w], axis=X)

            # ---- single ln over the chained product
            lns = wp.tile([P, w], bf16, tag="lns")
            nc.scalar.activation(lns[:], prod_prev[:], A.Ln,
                                 accum_out=acc_ln[:])

            # ---- final per-partition fold + store
            out_t = cp.tile([P, 4], f32)
            nc.vector.tensor_copy(out_t[:, 0:1], acc_ln[:])
            nc.vector.reduce_sum(out_t[:, 1:3], acc_b[:], axis=X)
            nc.vector.reduce_sum(out_t[:, 3:4], acc_sg[:], axis=X)
            nc.sync.dma_start(out_d[:], out_t[:])

    # Force a single activation table containing Exp+Ln+Sign so the
    # compiler does not ping-pong ACT_TABLE_LOADs.
    import concourse.bacc as bacc_mod
    from concourse.hw_specs import get_activation_tables
    orig = get_activation_tables(nc.m.arch)
    combined = None
    for k, v in orig.items():
        if (mybir.ActivationFunctionType.Exp in v
                and mybir.ActivationFunctionType.Ln in v
                and mybir.ActivationFunctionType.Sign in v):
            combined = k
            break
    if combined is not None:
        patched = {k: (v if k == combined else set()) for k, v in orig.items()}
        saved = bacc_mod.get_activation_tables
        bacc_mod.get_activation_tables = lambda arch: patched
        try:
            nc.compile()
        finally:
            bacc_mod.get_activation_tables = saved
    else:
        nc.compile()
    return nc


def _get_nc():
    if "nc" not in _CACHE:
        _CACHE["nc"] = _build_nc()
    return _CACHE["nc"]


# ------------------------------------------------------------------- host
def _host_prep(pred, target):
    """Class-bucketed shard/pack: bf16 tiles [P, 10, TILE_W] per core."""
    import ml_dtypes

    pred = np.asarray(pred)
    if pred.dtype != ml_dtypes.bfloat16:
        pred = pred.astype(np.float32).astype(ml_dtypes.bfloat16)
    target = np.asarray(target).astype(np.int32)

    order = np.argsort(target, kind="stable")
    counts = np.bincount(target, minlength=C)
    offs = np.zeros(C + 1, np.int64)
    offs[1:] = np.cumsum(counts)

    in_maps = []
    for k in range(N_CORES):
        R = np.full((C, BUCKET_COLS * P), -1, np.int64)
        for c in range(C):
            cnt = int(counts[c])
            base, rem = divmod(cnt, N_CORES)
            share = base + (1 if k < rem else 0)
            assert share <= BUCKET_COLS * P, (
                f"class {c} overflow on core {k}: {share}")
            start = offs[c] + k * base + min(k, rem)
            R[c, :share] = order[start:start + share]
        # [C, P*200] -> [C, P, 200] -> [P, C, 200] -> [P, W_CORE]
        Rpw = R.reshape(C, P, BUCKET_COLS).transpose(1, 0, 2)

        flat = Rpw.reshape(-1)
        Xg = pred[np.where(flat >= 0, flat, 0)]
        Xg[flat < 0] = ml_dtypes.bfloat16(0.0)
        # [P, C_bucket, 200, C_class] -> [P, C_class, C_bucket*200]
        Xc = Xg.reshape(P, C, BUCKET_COLS, C).transpose(0, 3, 1, 2) \
               .reshape(P, C, W_CORE)

        m = {}
        for i in range(N_TILES):
            sl = Xc[:, :, i * TILE_W:(i + 1) * TILE_W]
            m[f"comb{i}"] = np.ascontiguousarray(sl).reshape(P, C * TILE_W)
        in_maps.append(m)
    return in_maps


def kernel(pred, target):
    from concourse.bass_utils import run_bass_kernel_spmd

    nc = _get_nc()
    in_maps = _host_prep(pred, target)
    res = run_bass_kernel_spmd(nc, in_maps, core_ids=list(range(N_CORES)))

    s_ln = s_xt = s_sg = 0.0
    for k in range(N_CORES):
        o = res.results[k]["out"].astype(np.float64)
        s_ln += o[:, 0].sum()
        s_xt += o[:, 1].sum() + o[:, 2].sum()
        s_sg += o[:, 3].sum()

    # ln(prod of 5 all-zero pad rows' s=10) adds ln(10) per pad row; pads
    # have d=0 -> sign 0 -> each contributes 0.5 to (S + L)/2, cancelled by
    # the -0.5*N_PADS term folded into using N instead of L_total below.
    s_ln -= N_PADS * np.log(10.0)
    mism = 0.5 * s_sg + 0.5 * N

    ce = (s_ln - s_xt) / N
    bce = 100.0 * mism / N
    return np.float32(ce + bce)
